# revision 1
# baseline (speedup 1.0000x reference)
"""Trainium2 Bass kernel for nn_Part_Block (SE-style dynamic-weight CNN block).

Computation (per batch b):
    pooled = mean_hw x[b]                       (C,)
    hidden = silu(pooled @ fc1_w.T + fc1_b)     (128,)
    dw     = (hidden @ fc2_w.T + fc2_b)         (P*C,) -> (P, C)
    base   = x[b] * conv_w + conv_b             (C, H, W)
    out    = softmax_p( einsum('chw,pc->phw', base, dw) )

Sharding: data-parallel over batch across the 8 cores (4 batches/core),
no collectives.  The depthwise conv is folded into the dynamic weights:
    logits[p,hw] = sum_c x[c,hw] * (conv_w[c]*dw[p,c]) + beta[p]
    beta[p]      = sum_c conv_b[c]*dw[p,c]
so `base` is never materialized and x is read once.

Placement: this backend executes bass NEFFs with a large flat cost per
instruction (~35-70us), a ~10ns/element cost on vector/scalar-engine
ops, and a fast (BLAS-like) path only for matmuls with contiguous
operands.  The SE "squeeze" path (global pool + two tiny FCs, 0.13% of
the FLOPs) is therefore computed on the host in fp32, and the device
kernel does the heavy data-parallel part: the (2048 -> 4)
channel-weighted reduction over the full 151MB activation tensor as PE
matmuls (per batch: 16 K-tiles x 2 N-halves, contiguous operands), plus
the softmax over parts (Exp with per-part bias on ScalarE, ones-matmul
column sums, reciprocal, ones-matmul partition broadcast, multiply).
x is shipped as bf16 (halves the upload; logit error ~1e-4 relative).
"""

from contextlib import ExitStack

import ml_dtypes
import numpy as np

import concourse.bass as bass
import concourse.mybir as mybir
import concourse.tile as tile
from concourse import bacc
from concourse.bass_utils import run_bass_kernel_spmd

N_CORES = 8
B, C, H, W = 32, 2048, 24, 24
HW = H * W                      # 576
P = 4                           # parts
B_LOC = B // N_CORES            # 4 batches per core
NT = C // 128                   # 16 c-tiles
NS = 288                        # einsum N split (576 = 2*288)

F32 = mybir.dt.float32
BF16 = mybir.dt.bfloat16

_BUILD_CACHE: dict = {}


def _build(repeat: int = 1):
    """Build + compile the SPMD single-core program (same on all 8 cores)."""
    nc = bacc.Bacc(
        "TRN2", target_bir_lowering=False, debug=False, num_devices=N_CORES
    )
    xs = nc.dram_tensor("xs", [B_LOC, C, HW], BF16, kind="ExternalInput")
    # dwst[c', b*64 + t*4 + p] = conv_w[t*128+c'] * dw[b, p, t*128+c']
    dwst_d = nc.dram_tensor("dwst", [128, B_LOC * NT * P], BF16, kind="ExternalInput")
    # beta[p, b]
    beta_d = nc.dram_tensor("beta", [P, B_LOC], F32, kind="ExternalInput")
    ys = nc.dram_tensor("ys", [B_LOC, P, HW], F32, kind="ExternalOutput")

    with tile.TileContext(nc) as tc:
        with ExitStack() as ctx:
            const = ctx.enter_context(tc.tile_pool(name="const", bufs=1))
            data = ctx.enter_context(tc.tile_pool(name="data", bufs=1))
            psum = ctx.enter_context(tc.tile_pool(name="ps", bufs=1, space="PSUM"))

            ones4 = const.tile([P, 1], F32)
            nc.vector.memset(ones4[:], 1.0)
            ones1x4 = const.tile([1, P], F32)
            nc.vector.memset(ones1x4[:], 1.0)

            for _ in range(repeat):
                dwst = data.tile([128, B_LOC * NT * P], BF16)
                nc.sync.dma_start(dwst[:], dwst_d.ap())
                beta = data.tile([P, B_LOC], F32)
                nc.sync.dma_start(beta[:], beta_d.ap())
                # x_sb[p, (b*16+t)*576+f] = x[b, t*128+p, f]
                x_sb = data.tile([128, B_LOC * NT * HW], BF16)
                for b in range(B_LOC):
                    nc.sync.dma_start(
                        x_sb[:, b * NT * HW : (b + 1) * NT * HW],
                        xs.ap()[b].rearrange("(t p) f -> p t f", p=128),
                    )

                e_sb = data.tile([P, B_LOC * HW], F32)
                r_sb = data.tile([1, B_LOC * HW], F32)
                out_sb = data.tile([P, B_LOC * HW], F32)
                for b in range(B_LOC):
                    # ---- einsum: logits[p, hw] = sum_t dwsT_t.T @ x_t
                    ps_a = psum.tile([P, NS], F32)
                    ps_b = psum.tile([P, NS], F32)
                    xo = (b * NT) * HW
                    for t in range(NT):
                        lw = dwst[:, b * NT * P + t * P : b * NT * P + (t + 1) * P]
                        nc.tensor.matmul(
                            ps_a[:],
                            lhsT=lw,
                            rhs=x_sb[:, xo + t * HW : xo + t * HW + NS],
                            start=(t == 0),
                            stop=(t == NT - 1),
                        )
                        nc.tensor.matmul(
                            ps_b[:],
                            lhsT=lw,
                            rhs=x_sb[:, xo + t * HW + NS : xo + (t + 1) * HW],
                            start=(t == 0),
                            stop=(t == NT - 1),
                        )
                    # ---- softmax over p: e = exp(logits + beta)
                    eo = b * HW
                    nc.scalar.activation(
                        e_sb[:, eo : eo + NS],
                        ps_a[:],
                        mybir.ActivationFunctionType.Exp,
                        bias=beta[:, b : b + 1],
                    )
                    nc.scalar.activation(
                        e_sb[:, eo + NS : eo + HW],
                        ps_b[:],
                        mybir.ActivationFunctionType.Exp,
                        bias=beta[:, b : b + 1],
                    )
                    cs_a = psum.tile([1, NS], F32)
                    cs_b = psum.tile([1, NS], F32)
                    nc.tensor.matmul(
                        cs_a[:], lhsT=ones4[:], rhs=e_sb[:, eo : eo + NS],
                        start=True, stop=True,
                    )
                    nc.tensor.matmul(
                        cs_b[:], lhsT=ones4[:], rhs=e_sb[:, eo + NS : eo + HW],
                        start=True, stop=True,
                    )
                    nc.vector.reciprocal(r_sb[:, eo : eo + NS], cs_a[:])
                    nc.vector.reciprocal(r_sb[:, eo + NS : eo + HW], cs_b[:])
                    r4_a = psum.tile([P, NS], F32)
                    r4_b = psum.tile([P, NS], F32)
                    nc.tensor.matmul(
                        r4_a[:], lhsT=ones1x4[:], rhs=r_sb[0:1, eo : eo + NS],
                        start=True, stop=True,
                    )
                    nc.tensor.matmul(
                        r4_b[:], lhsT=ones1x4[:], rhs=r_sb[0:1, eo + NS : eo + HW],
                        start=True, stop=True,
                    )
                    nc.vector.tensor_mul(
                        out_sb[:, eo : eo + NS], e_sb[:, eo : eo + NS], r4_a[:]
                    )
                    nc.vector.tensor_mul(
                        out_sb[:, eo + NS : eo + HW], e_sb[:, eo + NS : eo + HW],
                        r4_b[:],
                    )
                nc.sync.dma_start(
                    ys.ap().rearrange("b p f -> p b f"), out_sb[:]
                )
    nc.compile()
    return nc


def _host_se(x3, fc1_w, fc1_b, fc2_w, fc2_b, conv_w, conv_b):
    """SE squeeze path on host (fp32/fp64, tiny): returns dwst (B,128,64) bf16
    and betaT (B, P) f32."""
    pooled = x3.mean(axis=2, dtype=np.float64)                    # (B, C)
    z = pooled @ fc1_w.astype(np.float64).T + fc1_b.astype(np.float64)
    hidden = z / (1.0 + np.exp(-z))                               # silu, (B, 128)
    dw = hidden @ fc2_w.astype(np.float64).T + fc2_b.astype(np.float64)  # (B, P*C)
    dws = dw * np.tile(conv_w.astype(np.float64), P)              # conv_w folded
    # dwst[b][c', t*4+p] = dws[b, p*C + t*128 + c']
    dwst = np.ascontiguousarray(
        dws.reshape(B, P, NT, 128).transpose(0, 3, 2, 1).reshape(B, 128, NT * P)
    ).astype(ml_dtypes.bfloat16)
    beta = dw.reshape(B, P, C) @ conv_b.astype(np.float64)        # (B, P)
    return dwst, beta.astype(np.float32)


def _run(in_maps, repeat: int = 1):
    if repeat not in _BUILD_CACHE:
        _BUILD_CACHE[repeat] = _build(repeat)
    nc = _BUILD_CACHE[repeat]
    return run_bass_kernel_spmd(nc, in_maps, list(range(N_CORES)))


def make_in_maps(x, fc1_w, fc1_b, fc2_w, fc2_b, conv_w, conv_b):
    x = np.asarray(x, np.float32)
    x3 = x.reshape(B, C, HW)
    xbf = x3.astype(ml_dtypes.bfloat16)
    dwst, beta = _host_se(
        x3,
        np.asarray(fc1_w, np.float32), np.asarray(fc1_b, np.float32),
        np.asarray(fc2_w, np.float32), np.asarray(fc2_b, np.float32),
        np.asarray(conv_w, np.float32), np.asarray(conv_b, np.float32),
    )
    in_maps = []
    for i in range(N_CORES):
        sl = slice(i * B_LOC, (i + 1) * B_LOC)
        in_maps.append({
            "xs": xbf[sl],
            "dwst": np.ascontiguousarray(
                dwst[sl].transpose(1, 0, 2).reshape(128, B_LOC * NT * P)
            ),
            "beta": np.ascontiguousarray(beta[sl].T),
        })
    return in_maps


def kernel(x, fc1_w, fc1_b, fc2_w, fc2_b, conv_w, conv_b):
    in_maps = make_in_maps(x, fc1_w, fc1_b, fc2_w, fc2_b, conv_w, conv_b)
    res = _run(in_maps, repeat=1)
    out = np.concatenate(
        [res.results[i]["ys"] for i in range(N_CORES)], axis=0
    )
    return np.ascontiguousarray(out.reshape(B, P, H, W).astype(np.float32))

